# revision 6
# baseline (speedup 1.0000x reference)
"""Trainium2 Bass kernel for CombinedGCNMLPModel (gnn_message_passing).

Strategy (8-core SPMD, data-parallel over graphs):
- Nodes repacked host-side into fixed-stride slots: graph g gets S_g=64 slots,
  64 graphs per core -> NPAD=4096 slots/core, 32768 global table rows. This
  makes the per-graph readout segments static across all 8 cores (one SPMD
  program), with zero-padded slots masked out of BN stats / readout.
- GCN layer: segsum(h[src]) computed in input space (linearity), then @ W.
  Scatter-sum via one-hot matrices (is_equal against an iota row) x TensorE
  matmul accumulation in PSUM, edges laid out host-side in 128-edge chunks.
- L1 edge values are host-pregathered from node_feats (input marshaling) and
  streamed as a dense table; L2 edge values are gathered on-device from the
  AllGathered h1 table with InstDMAGatherAnt (1024 rows/instruction).
- Cross-core: AllReduce for BN stats (2KB), AllGather for the h1 gather table.
- BatchNorm / biases / readout run in transposed layout [chan, nodes] so all
  per-channel terms are per-partition scalars.
- rdkit MLP + combined head run per-core on its 64 graphs with replicated
  bf16 weights, fully overlapped with the GCN phases.
"""
import numpy as np
import ml_dtypes

import concourse.bacc as bacc
import concourse.bass as bass
import concourse.mybir as mybir
import concourse.tile as tile
from concourse.bass_utils import run_bass_kernel_spmd

BF16 = mybir.dt.bfloat16
F32 = mybir.dt.float32
I16 = mybir.dt.int16
bf = ml_dtypes.bfloat16

NCORES = 8
N_GRAPHS = 512
GPC = N_GRAPHS // NCORES  # 64 graphs per core
N_NODES = 20000
N_EDGES = 320000
IN_F = 128
HID = 256
RDKIT_IN = 2048
RDK_H1, RDK_H2 = 1024, 512
COMB_H1, COMB_H2 = 1024, 512
EPS = 1e-5

TRACE = False
DEBUG = False
LAST_EXEC_NS = None

_prog_cache = {}


def _bfc(x):
    return np.ascontiguousarray(np.asarray(x, np.float32)).astype(bf)


def _tile_weight(w, km, mm):
    """[K, M] -> [n_m*n_k, 128, 128] with t = m*n_k + k."""
    K, M = w.shape
    nk, nm = K // km, M // mm
    t = w.reshape(nk, km, nm, mm).transpose(2, 0, 1, 3).reshape(nm * nk, km, mm)
    return np.ascontiguousarray(t)


def _host_prep(node_feats, rdkit_feats, src, dst, node_gid, weights):
    gid = np.asarray(node_gid)
    src = np.asarray(src).astype(np.int64)
    dst = np.asarray(dst).astype(np.int64)
    h0 = np.asarray(node_feats, np.float32)

    sizes = np.bincount(gid, minlength=N_GRAPHS)
    max_sz = int(sizes.max())
    Sg = int(np.ceil((max_sz + 1) / 8) * 8)
    assert Sg <= 64, f"graph too large for int16 table: max={max_sz}"
    Sg = max(Sg, 8)
    NPAD = GPC * Sg
    ROWS = NCORES * NPAD
    assert ROWS <= 32768

    starts = np.concatenate([[0], np.cumsum(sizes)])
    rank = np.arange(N_NODES) - starts[gid]  # gid is sorted
    core_of = (gid // GPC).astype(np.int64)
    slot_local = (gid % GPC).astype(np.int64) * Sg + rank
    slot_global = core_of * NPAD + slot_local

    ecore = core_of[dst]
    ed = slot_local[dst]  # local dst slot
    esrow = slot_global[src]  # L2 gather row

    # chunk counts (static across cores)
    t128 = ecore * (NPAD // 128) + ed // 128
    CH1 = int(np.ceil(np.bincount(t128, minlength=NCORES * NPAD // 128).max() / 128))
    g512 = ecore * (NPAD // 512) + ed // 512
    CH2 = int(np.ceil(np.bincount(g512, minlength=NCORES * NPAD // 512).max() / 128))
    # L2 stream length must be a multiple of 1024 (8 chunks/gather, NGRP=8)
    NGRP = NPAD // 512
    NT1 = NPAD // 128
    NC1 = NT1 * CH1
    NC2 = NGRP * CH2
    assert (NC2 * 128) % 1024 == 0

    h0b = _bfc(h0)
    zero_row_l2 = Sg - 1  # pad slot of core0/graph0

    per_core = []
    for c in range(NCORES):
        em = np.nonzero(ecore == c)[0]
        es_c, ed_c, erow_c = src[em], ed[em], esrow[em]

        # ---- L1 stream: 128-slot tiles, CH1 chunks each
        n1 = NC1 * 128
        src1 = np.full(n1, -1, np.int64)
        dl1 = np.zeros(n1, np.int16)
        t1 = ed_c // 128
        for t in range(NT1):
            sel = np.nonzero(t1 == t)[0]
            base = t * CH1 * 128
            assert len(sel) <= CH1 * 128
            src1[base : base + len(sel)] = es_c[sel]
            dl1[base : base + len(sel)] = (ed_c[sel] % 128).astype(np.int16)
        g1 = np.zeros((n1, IN_F), bf)
        v = src1 >= 0
        g1[v] = h0b[src1[v]]
        g1tab = np.ascontiguousarray(
            g1.reshape(NC1, 128, IN_F).transpose(1, 0, 2).reshape(128, NC1 * IN_F)
        )
        dl1_t = np.ascontiguousarray(dl1.reshape(NC1, 128).T)

        # ---- L2 stream: 512-slot groups, CH2 chunks each, sorted by src row
        n2 = NC2 * 128
        row2 = np.full(n2, zero_row_l2, np.int64)
        dl2 = np.zeros(n2, np.int16)
        gg = ed_c // 512
        for g in range(NGRP):
            sel = np.nonzero(gg == g)[0]
            sel = sel[np.argsort(erow_c[sel], kind="stable")]
            base = g * CH2 * 128
            assert len(sel) <= CH2 * 128
            row2[base : base + len(sel)] = erow_c[sel]
            dl2[base : base + len(sel)] = (ed_c[sel] % 512).astype(np.int16)
        assert row2.max() < 32768
        sx2 = np.tile(row2.astype(np.int16).reshape(-1, 16).T, (8, 1))
        sx2 = np.ascontiguousarray(sx2)
        dl2_t = np.ascontiguousarray(dl2.reshape(NC2, 128).T)

        # ---- per-core slabs
        nm = np.nonzero(core_of == c)[0]
        h0T = np.zeros((IN_F, NPAD), bf)
        h0T[:, slot_local[nm]] = h0b[nm].T
        mask = np.zeros((1, NPAD), np.float32)
        mask[0, slot_local[nm]] = 1.0
        mask_t = np.ascontiguousarray(np.broadcast_to(mask, (128, NPAD))).astype(bf)
        negm = np.where(mask > 0, 0.0, -1e30).astype(np.float32)
        negm_t = np.ascontiguousarray(np.broadcast_to(negm, (128, NPAD))).astype(bf)

        rdkT = _bfc(np.asarray(rdkit_feats)[c * GPC : (c + 1) * GPC].T)  # [2048, 64]
        rdk_tiles = np.ascontiguousarray(rdkT.reshape(16, 128, GPC))

        per_core.append(
            dict(
                g1tab=g1tab,
                dl1=dl1_t,
                dl2=dl2_t,
                sx2=sx2,
                h0T=np.ascontiguousarray(h0T),
                mask=mask_t,
                negmask=negm_t,
                rdkT=rdk_tiles,
            )
        )

    # ---- shared (replicated) arrays
    W = weights
    vecs = np.zeros((128, 48), np.float32)

    def setcol(j, v):
        vecs[: len(v), j] = v

    for m in range(2):
        setcol(0 + m, np.asarray(W["b1"], np.float32)[m * 128 : (m + 1) * 128])
        setcol(2 + m, np.asarray(W["rb1"], np.float32)[m * 128 : (m + 1) * 128])
        setcol(4 + m, np.asarray(W["g1"], np.float32)[m * 128 : (m + 1) * 128])
        setcol(6 + m, np.asarray(W["be1"], np.float32)[m * 128 : (m + 1) * 128])
        setcol(8 + m, np.asarray(W["b2"], np.float32)[m * 128 : (m + 1) * 128])
        setcol(10 + m, np.asarray(W["rb2"], np.float32)[m * 128 : (m + 1) * 128])
        setcol(12 + m, np.asarray(W["g2"], np.float32)[m * 128 : (m + 1) * 128])
        setcol(14 + m, np.asarray(W["be2"], np.float32)[m * 128 : (m + 1) * 128])
    vecs[0, 16] = float(np.asarray(W["aw_b"]).reshape(-1)[0])
    vecs[0, 17] = float(np.asarray(W["c_b3"]).reshape(-1)[0])
    for m in range(8):
        setcol(18 + m, np.asarray(W["rk_b1"], np.float32)[m * 128 : (m + 1) * 128])
    for m in range(4):
        setcol(26 + m, np.asarray(W["rk_b2"], np.float32)[m * 128 : (m + 1) * 128])
    for m in range(8):
        setcol(30 + m, np.asarray(W["c_b1"], np.float32)[m * 128 : (m + 1) * 128])
    for m in range(4):
        setcol(38 + m, np.asarray(W["c_b2"], np.float32)[m * 128 : (m + 1) * 128])

    iota128 = np.ascontiguousarray(
        np.tile(np.arange(128, dtype=np.int16), (128, 1))
    )
    iota512 = np.ascontiguousarray(
        np.tile(np.arange(512, dtype=np.int16), (128, 1))
    )
    ident = np.eye(128, dtype=np.float32).astype(bf)

    shared = dict(
        W1=_bfc(W["W1"]),  # [128, 256]
        rW1=_bfc(W["rW1"]),
        W2=np.ascontiguousarray(_bfc(W["W2"]).reshape(2, 128, 256)),
        rW2=np.ascontiguousarray(_bfc(W["rW2"]).reshape(2, 128, 256)),
        awW=np.ascontiguousarray(_bfc(W["aw_W"]).reshape(2, 128).T),  # [128, 2]
        rkW1=_tile_weight(_bfc(W["rk_W1"]), 128, 128),  # [128,128,128] m*16+k
        rkW2=_tile_weight(_bfc(W["rk_W2"]), 128, 128),  # [32,...]
        cW1=_tile_weight(_bfc(W["c_W1"]), 128, 128),  # [64,...]
        cW2=_tile_weight(_bfc(W["c_W2"]), 128, 128),  # [32,...]
        cW3=np.ascontiguousarray(_bfc(W["c_W3"]).reshape(4, 128).T),  # [128, 4]
        vecs=vecs,
        iota128=iota128,
        iota512=iota512,
        ident=ident,
    )
    cfg = dict(Sg=Sg, NPAD=NPAD, ROWS=ROWS, CH1=CH1, CH2=CH2, NC1=NC1, NC2=NC2,
               NT1=NT1, NGRP=NGRP)
    return cfg, per_core, shared


def _build(cfg, debug):
    NPAD, ROWS = cfg["NPAD"], cfg["ROWS"]
    CH1, CH2 = cfg["CH1"], cfg["CH2"]
    NC1, NC2 = cfg["NC1"], cfg["NC2"]
    NT1, NGRP = cfg["NT1"], cfg["NGRP"]
    Sg = cfg["Sg"]
    NSEG = NPAD // Sg  # graphs per core
    NGATH = NC2 * 128 // 1024

    nc = bacc.Bacc(
        "TRN2",
        target_bir_lowering=False,
        debug=False,
        enable_asserts=False,
        num_devices=NCORES,
    )

    def din(name, shape, dt):
        return nc.dram_tensor(name, shape, dt, kind="ExternalInput")

    g1tab = din("g1tab", [128, NC1 * IN_F], BF16)
    dl1 = din("dl1", [128, NC1], I16)
    dl2 = din("dl2", [128, NC2], I16)
    sx2 = din("sx2", [128, NC2 * 8], I16)
    h0T = din("h0T", [128, NPAD], BF16)
    maskD = din("mask", [128, NPAD], BF16)
    negmD = din("negmask", [128, NPAD], BF16)
    rdkD = din("rdkT", [16, 128, GPC], BF16)
    W1D = din("W1", [128, 256], BF16)
    rW1D = din("rW1", [128, 256], BF16)
    W2D = din("W2", [2, 128, 256], BF16)
    rW2D = din("rW2", [2, 128, 256], BF16)
    awWD = din("awW", [128, 2], BF16)
    rkW1D = din("rkW1", [128, 128, 128], BF16)
    rkW2D = din("rkW2", [32, 128, 128], BF16)
    cW1D = din("cW1", [64, 128, 128], BF16)
    cW2D = din("cW2", [32, 128, 128], BF16)
    cW3D = din("cW3", [128, 4], BF16)
    vecsD = din("vecs", [128, 48], F32)
    iota128D = din("iota128", [128, 128], I16)
    iota512D = din("iota512", [128, 512], I16)
    identD = din("ident", [128, 128], BF16)

    outD = nc.dram_tensor("out", [1, GPC], F32, kind="ExternalOutput")
    dbg = {}
    if debug:
        for nm, shp, dt in [
            ("agg1dbg", [128, NPAD], BF16),
            ("new1dbg", [128, NPAD], BF16),
            ("h1dbg", [128, NPAD], BF16),
            ("agg2dbg", [128, NPAD], BF16),
            ("h2dbg", [128, NPAD], BF16),
            ("wdbg", [1, NPAD], BF16),
            ("zdbg", [128, 8 * GPC], BF16),
            ("st1dbg", [128, 4], F32),
        ]:
            dbg[nm] = nc.dram_tensor(nm, shp, dt, kind="ExternalOutput")

    h1_shard = nc.dram_tensor("h1_shard", [NPAD, HID], BF16)
    h1_full = nc.dram_tensor("h1_full", [ROWS, HID], BF16, addr_space="Shared")
    st1_in = nc.dram_tensor("st1_in", [128, 4], F32)
    st1_out = nc.dram_tensor("st1_out", [128, 4], F32, addr_space="Shared")
    st2_in = nc.dram_tensor("st2_in", [128, 4], F32)
    st2_out = nc.dram_tensor("st2_out", [128, 4], F32, addr_space="Shared")

    RG = [list(range(NCORES))]
    Relu = mybir.ActivationFunctionType.Relu
    Copy = mybir.ActivationFunctionType.Copy
    Sigmoid = mybir.ActivationFunctionType.Sigmoid
    Sqrt = mybir.ActivationFunctionType.Sqrt
    Square = mybir.ActivationFunctionType.Square
    AL = mybir.AluOpType
    AX = mybir.AxisListType.X

    with tile.TileContext(nc) as tc:
        with (
            tc.tile_pool(name="persist", bufs=1) as pp,
            tc.tile_pool(name="g1p", bufs=2) as g1p,
            tc.tile_pool(name="s1p", bufs=2) as s1p,
            tc.tile_pool(name="g2p", bufs=2) as g2p,
            tc.tile_pool(name="s2p", bufs=2) as s2p,
            tc.tile_pool(name="wp", bufs=4) as wp,
            tc.tile_pool(name="scr", bufs=2) as scr,
            tc.tile_pool(name="trp", bufs=2) as trp,
            tc.tile_pool(name="p_agg1", bufs=1, space="PSUM") as p_agg1,
            tc.tile_pool(name="p_agg2", bufs=1, space="PSUM") as p_agg2,
            tc.tile_pool(name="p_w", bufs=2, space="PSUM") as p_w,
            tc.tile_pool(name="p_tr", bufs=1, space="PSUM") as p_tr,
            tc.tile_pool(name="p_sm", bufs=2, space="PSUM") as p_sm,
        ):
            # ---------- constant loads ----------
            def load(name, dram, shape, dt):
                t = pp.tile(shape, dt, tag=name, name=name)
                nc.sync.dma_start(out=t[:], in_=dram[tuple(slice(None) for _ in shape)])
                return t

            dl1_t = load("dl1", dl1, [128, NC1], I16)
            dl2_t = load("dl2", dl2, [128, NC2], I16)
            sx2_t = load("sx2", sx2, [128, NC2 * 8], I16)
            h0T_t = load("h0T", h0T, [128, NPAD], BF16)
            mask_t = pp.tile([128, NPAD], BF16, tag="mask8k", name="mask")
            nc.sync.dma_start(out=mask_t[:], in_=maskD[:, :])
            iota128_t = load("iota128", iota128D, [128, 128], I16)
            iota512_t = load("iota512", iota512D, [128, 512], I16)
            ident_t = load("ident", identD, [128, 128], BF16)
            W1_t = load("W1", W1D, [128, 256], BF16)
            rW1_t = load("rW1", rW1D, [128, 256], BF16)
            vecs_t = load("vecs", vecsD, [128, 48], F32)
            awW_t = load("awW", awWD, [128, 2], BF16)
            cW3_t = load("cW3", cW3D, [128, 4], BF16)
            W2_t, rW2_t, rdk_t = [], [], []
            for k in range(2):
                t = pp.tile([128, 256], BF16, tag=f"W2_{k}", name=f"W2_{k}")
                nc.sync.dma_start(out=t[:], in_=W2D[k, :, :])
                W2_t.append(t)
                t = pp.tile([128, 256], BF16, tag=f"rW2_{k}", name=f"rW2_{k}")
                nc.sync.dma_start(out=t[:], in_=rW2D[k, :, :])
                rW2_t.append(t)
            for k in range(16):
                t = pp.tile([128, GPC], BF16, tag=f"rdk_{k}", name=f"rdk_{k}")
                nc.sync.dma_start(out=t[:], in_=rdkD[k, :, :])
                rdk_t.append(t)

            aggH1 = pp.tile([128, NPAD], BF16, tag="shscr8k", name="aggH1")
            aggH2 = [pp.tile([128, NPAD], BF16, tag=f"aggH2_{h}", name=f"aggH2_{h}") for h in range(2)]
            new_t = [pp.tile([128, NPAD], BF16, tag=f"new_{m}", name=f"new_{m}") for m in range(2)]
            h1T = [pp.tile([128, NPAD], BF16, tag=f"h1T_{m}", name=f"h1T_{m}") for m in range(2)]
            h2T = [pp.tile([128, NPAD], BF16, tag=f"h2T_{m}", name=f"h2T_{m}") for m in range(2)]
            st_sb = pp.tile([128, 4], F32, tag="st_sb")
            stt_sb = pp.tile([128, 4], F32, tag="stt_sb")
            st2_sb = pp.tile([128, 4], F32, tag="st2_sb")
            stt2_sb = pp.tile([128, 4], F32, tag="stt2_sb")
            wb_row = pp.tile([1, NPAD], BF16, tag="wb_row")
            ones1 = pp.tile([1, 128], BF16, tag="ones1")
            nc.vector.memset(ones1[:], 1.0)
            zcol = pp.tile([128, 1], F32, tag="zcol")
            nc.vector.memset(zcol[:], 0.0)
            z_t = [pp.tile([128, GPC], BF16, tag=f"z_{i}", name=f"z_{i}") for i in range(8)]
            c1_t = [pp.tile([128, GPC], BF16, tag=f"c1_{i}", name=f"c1_{i}") for i in range(8)]
            c2_t = [pp.tile([128, GPC], BF16, tag=f"c2_{i}", name=f"c2_{i}") for i in range(4)]

            # ---------- phase 1: L1 edge aggregation ----------
            for t in range(NT1):
                g1 = g1p.tile([128, CH1 * IN_F], BF16, tag="g1")
                nc.sync.dma_start(
                    out=g1[:], in_=g1tab[:, t * CH1 * IN_F : (t + 1) * CH1 * IN_F]
                )
                s1 = s1p.tile([128, CH1 * 128], BF16, tag="s1")
                nc.vector.tensor_tensor(
                    out=s1[:].rearrange("p (c j) -> p c j", j=128),
                    in0=iota128_t[:].unsqueeze(1).broadcast_to([128, CH1, 128]),
                    in1=dl1_t[:, t * CH1 : (t + 1) * CH1]
                    .unsqueeze(2)
                    .broadcast_to([128, CH1, 128]),
                    op=AL.is_equal,
                )
                ps = p_agg1.tile([128, 128], F32, tag="agg1")
                for c in range(CH1):
                    nc.tensor.matmul(
                        out=ps[:],
                        lhsT=g1[:, c * IN_F : (c + 1) * IN_F],
                        rhs=s1[:, c * 128 : (c + 1) * 128],
                        start=(c == 0),
                        stop=(c == CH1 - 1),
                    )
                nc.scalar.activation(
                    out=aggH1[:, t * 128 : (t + 1) * 128], in_=ps[:], func=Copy
                )

            # ---------- phase 2: rdkit MLP (independent; fills gaps) ----------
            r1_t = []
            for m in range(8):
                ps = p_sm.tile([128, GPC], F32, tag="sm")
                for k in range(16):
                    wt = wp.tile([128, 128], BF16, tag="wt")
                    nc.sync.dma_start(out=wt[:], in_=rkW1D[m * 16 + k, :, :])
                    nc.tensor.matmul(
                        out=ps[:], lhsT=wt[:], rhs=rdk_t[k][:],
                        start=(k == 0), stop=(k == 15),
                    )
                r1 = pp.tile([128, GPC], BF16, tag=f"r1_{m}", name=f"r1_{m}")
                nc.scalar.activation(
                    out=r1[:], in_=ps[:], func=Relu, bias=vecs_t[:, 18 + m : 19 + m]
                )
                r1_t.append(r1)
            for m in range(4):
                ps = p_sm.tile([128, GPC], F32, tag="sm")
                for k in range(8):
                    wt = wp.tile([128, 128], BF16, tag="wt")
                    nc.sync.dma_start(out=wt[:], in_=rkW2D[m * 8 + k, :, :])
                    nc.tensor.matmul(
                        out=ps[:], lhsT=wt[:], rhs=r1_t[k][:],
                        start=(k == 0), stop=(k == 7),
                    )
                nc.scalar.activation(
                    out=z_t[4 + m][:], in_=ps[:], func=Relu,
                    bias=vecs_t[:, 26 + m : 27 + m],
                )

            # ---------- phase 3: L1 linear + BN ----------
            for n in range(NPAD // 512):
                sl = slice(n * 512, (n + 1) * 512)
                for m in range(2):
                    msl = slice(m * 128, (m + 1) * 128)
                    ps_a = p_w.tile([128, 512], F32, tag="wps")
                    nc.tensor.matmul(
                        out=ps_a[:], lhsT=W1_t[:, msl], rhs=aggH1[:, sl],
                        start=True, stop=True,
                    )
                    t1 = scr.tile([128, 512], F32, tag="relu_a")
                    nc.scalar.activation(
                        out=t1[:], in_=ps_a[:], func=Relu, bias=vecs_t[:, m : m + 1]
                    )
                    ps_r = p_w.tile([128, 512], F32, tag="wps")
                    nc.tensor.matmul(
                        out=ps_r[:], lhsT=rW1_t[:, msl], rhs=h0T_t[:, sl],
                        start=True, stop=True,
                    )
                    t2 = scr.tile([128, 512], F32, tag="relu_b")
                    nc.scalar.activation(
                        out=t2[:], in_=ps_r[:], func=Relu, bias=vecs_t[:, 2 + m : 3 + m]
                    )
                    nc.vector.tensor_tensor(
                        out=new_t[m][:, sl], in0=t1[:], in1=t2[:], op=AL.add
                    )
            if debug:
                nc.sync.dma_start(out=dbg["agg1dbg"][:, :], in_=aggH1[:])
            sqscr3 = pp.tile([128, NPAD], BF16, tag="shscr8k", name="sqscr3")
            for m in range(2):
                nc.vector.tensor_tensor(
                    out=new_t[m][:], in0=new_t[m][:], in1=mask_t[:], op=AL.mult
                )
                nc.vector.reduce_sum(
                    out=st_sb[:, 2 * m : 2 * m + 1], in_=new_t[m][:], axis=AX
                )
                nc.scalar.activation(
                    out=sqscr3[:], in_=new_t[m][:], func=Square,
                    accum_out=st_sb[:, 2 * m + 1 : 2 * m + 2],
                )
            nc.sync.dma_start(out=st1_in[:, :], in_=st_sb[:])
            nc.gpsimd.collective_compute(
                "AllReduce", AL.add, replica_groups=RG,
                ins=[st1_in[:, :]], outs=[st1_out[:, :]],
            )
            nc.sync.dma_start(out=stt_sb[:], in_=st1_out[:, :])
            if debug:
                nc.sync.dma_start(out=dbg["st1dbg"][:, :], in_=stt_sb[:])

            def bn_coefs(stt, m, g_col, be_col, tag):
                mu = scr.tile([128, 1], F32, tag=f"{tag}mu", name=f"{tag}mu")
                nc.vector.tensor_scalar(
                    out=mu[:], in0=stt[:, 2 * m : 2 * m + 1], scalar1=1.0 / N_NODES,
                    scalar2=None, op0=AL.mult,
                )
                ex2 = scr.tile([128, 1], F32, tag=f"{tag}ex2", name=f"{tag}ex2")
                nc.vector.tensor_scalar(
                    out=ex2[:], in0=stt[:, 2 * m + 1 : 2 * m + 2],
                    scalar1=1.0 / N_NODES, scalar2=None, op0=AL.mult,
                )
                mu2 = scr.tile([128, 1], F32, tag=f"{tag}mu2", name=f"{tag}mu2")
                nc.vector.tensor_tensor(out=mu2[:], in0=mu[:], in1=mu[:], op=AL.mult)
                var = scr.tile([128, 1], F32, tag=f"{tag}var", name=f"{tag}var")
                nc.vector.tensor_tensor(out=var[:], in0=ex2[:], in1=mu2[:], op=AL.subtract)
                nc.vector.tensor_scalar(
                    out=var[:], in0=var[:], scalar1=float(EPS), scalar2=None,
                    op0=AL.add,
                )
                sd = scr.tile([128, 1], F32, tag=f"{tag}sd", name=f"{tag}sd")
                nc.scalar.activation(out=sd[:], in_=var[:], func=Sqrt, bias=zcol[:, :1])
                rin = scr.tile([128, 1], F32, tag=f"{tag}rin", name=f"{tag}rin")
                nc.vector.reciprocal(out=rin[:], in_=sd[:])
                al = scr.tile([128, 1], F32, tag=f"{tag}al", name=f"{tag}al")
                nc.vector.tensor_tensor(
                    out=al[:], in0=rin[:], in1=vecs_t[:, g_col + m : g_col + m + 1],
                    op=AL.mult,
                )
                amu = scr.tile([128, 1], F32, tag=f"{tag}amu", name=f"{tag}amu")
                nc.vector.tensor_tensor(out=amu[:], in0=al[:], in1=mu[:], op=AL.mult)
                be = scr.tile([128, 1], F32, tag=f"{tag}be", name=f"{tag}be")
                nc.vector.tensor_tensor(
                    out=be[:], in0=vecs_t[:, be_col + m : be_col + m + 1], in1=amu[:],
                    op=AL.subtract,
                )
                return al, be

            for m in range(2):
                al, be = bn_coefs(stt_sb, m, 4, 6, f"bn1_{m}")
                nc.vector.tensor_scalar(
                    out=h1T[m][:], in0=new_t[m][:], scalar1=al[:, :1],
                    scalar2=be[:, :1], op0=AL.mult, op1=AL.add,
                )
                nc.vector.tensor_tensor(
                    out=h1T[m][:], in0=h1T[m][:], in1=mask_t[:], op=AL.mult
                )

            if debug:
                nc.sync.dma_start(out=dbg["new1dbg"][:, :], in_=new_t[0][:])
                nc.sync.dma_start(out=dbg["h1dbg"][:, :], in_=h1T[0][:])

            # ---------- phase 4: h1 shard transpose + AllGather ----------
            for t in range(NT1):
                h1n = trp.tile([128, HID], BF16, tag="h1n")
                for m in range(2):
                    pt = p_tr.tile([128, 128], BF16, tag="tr")
                    nc.tensor.transpose(
                        out=pt[:], in_=h1T[m][:, t * 128 : (t + 1) * 128],
                        identity=ident_t[:],
                    )
                    nc.scalar.activation(
                        out=h1n[:, m * 128 : (m + 1) * 128], in_=pt[:], func=Copy
                    )
                nc.sync.dma_start(
                    out=h1_shard[t * 128 : (t + 1) * 128, :], in_=h1n[:]
                )
            nc.gpsimd.collective_compute(
                "AllGather", AL.bypass, replica_groups=RG,
                ins=[h1_shard[:, :]], outs=[h1_full[:, :]],
            )

            # ---------- phase 5: L2 edge aggregation (device gather) ----------
            ps2 = None
            for gi in range(NGATH):
                g2 = g2p.tile([128, 8 * HID], BF16, tag="g2")
                nc.gpsimd.dma_gather(
                    g2[:].rearrange("p (c e) -> p c e", e=HID),
                    h1_full[:, :],
                    sx2_t[:, gi * 64 : (gi + 1) * 64],
                    1024,
                    1024,
                    HID,
                )
                s2 = s2p.tile([128, 8 * 512], BF16, tag="s2")
                nc.vector.tensor_tensor(
                    out=s2[:].rearrange("p (c j) -> p c j", j=512),
                    in0=iota512_t[:].unsqueeze(1).broadcast_to([128, 8, 512]),
                    in1=dl2_t[:, gi * 8 : (gi + 1) * 8]
                    .unsqueeze(2)
                    .broadcast_to([128, 8, 512]),
                    op=AL.is_equal,
                )
                for c8 in range(8):
                    ch = gi * 8 + c8
                    grp, ci = divmod(ch, CH2)
                    if ci == 0:
                        ps2 = [
                            p_agg2.tile([128, 512], F32, tag=f"agg2_{h}", name=f"agg2ps_{h}")
                            for h in range(2)
                        ]
                    for h in range(2):
                        nc.tensor.matmul(
                            out=ps2[h][:],
                            lhsT=g2[:, c8 * HID + h * 128 : c8 * HID + (h + 1) * 128],
                            rhs=s2[:, c8 * 512 : (c8 + 1) * 512],
                            start=(ci == 0),
                            stop=(ci == CH2 - 1),
                        )
                    if ci == CH2 - 1:
                        for h in range(2):
                            nc.scalar.activation(
                                out=aggH2[h][:, grp * 512 : (grp + 1) * 512],
                                in_=ps2[h][:], func=Copy,
                            )

            # ---------- phase 6: L2 linear + BN ----------
            for n in range(NPAD // 512):
                sl = slice(n * 512, (n + 1) * 512)
                for m in range(2):
                    msl = slice(m * 128, (m + 1) * 128)
                    ps_a = p_w.tile([128, 512], F32, tag="wps")
                    for k in range(2):
                        nc.tensor.matmul(
                            out=ps_a[:], lhsT=W2_t[k][:, msl], rhs=aggH2[k][:, sl],
                            start=(k == 0), stop=(k == 1),
                        )
                    t1 = scr.tile([128, 512], F32, tag="relu_a")
                    nc.scalar.activation(
                        out=t1[:], in_=ps_a[:], func=Relu, bias=vecs_t[:, 8 + m : 9 + m]
                    )
                    ps_r = p_w.tile([128, 512], F32, tag="wps")
                    for k in range(2):
                        nc.tensor.matmul(
                            out=ps_r[:], lhsT=rW2_t[k][:, msl], rhs=h1T[k][:, sl],
                            start=(k == 0), stop=(k == 1),
                        )
                    t2 = scr.tile([128, 512], F32, tag="relu_b")
                    nc.scalar.activation(
                        out=t2[:], in_=ps_r[:], func=Relu,
                        bias=vecs_t[:, 10 + m : 11 + m],
                    )
                    nc.vector.tensor_tensor(
                        out=new_t[m][:, sl], in0=t1[:], in1=t2[:], op=AL.add
                    )
            sqscr6 = pp.tile([128, NPAD], BF16, tag="shscr8k", name="sqscr6")
            for m in range(2):
                nc.vector.tensor_tensor(
                    out=new_t[m][:], in0=new_t[m][:], in1=mask_t[:], op=AL.mult
                )
                nc.vector.reduce_sum(
                    out=st2_sb[:, 2 * m : 2 * m + 1], in_=new_t[m][:], axis=AX
                )
                nc.scalar.activation(
                    out=sqscr6[:], in_=new_t[m][:], func=Square,
                    accum_out=st2_sb[:, 2 * m + 1 : 2 * m + 2],
                )
            nc.sync.dma_start(out=st2_in[:, :], in_=st2_sb[:])
            nc.gpsimd.collective_compute(
                "AllReduce", AL.add, replica_groups=RG,
                ins=[st2_in[:, :]], outs=[st2_out[:, :]],
            )
            nc.sync.dma_start(out=stt2_sb[:], in_=st2_out[:, :])
            for m in range(2):
                al, be = bn_coefs(stt2_sb, m, 12, 14, f"bn2_{m}")
                nc.vector.tensor_scalar(
                    out=h2T[m][:], in0=new_t[m][:], scalar1=al[:, :1],
                    scalar2=be[:, :1], op0=AL.mult, op1=AL.add,
                )
                nc.vector.tensor_tensor(
                    out=h2T[m][:], in0=h2T[m][:], in1=mask_t[:], op=AL.mult
                )

            # ---------- phase 7: readout ----------
            negm_t = pp.tile([128, NPAD], BF16, tag="mask8k", name="negm")
            nc.sync.dma_start(out=negm_t[:], in_=negmD[:, :])
            hw_t = pp.tile([128, NPAD], BF16, tag="shscr8k", name="hw")
            for n in range(NPAD // 512):
                sl = slice(n * 512, (n + 1) * 512)
                psw = p_sm.tile([1, 512], F32, tag="sm", name="psw")
                for k in range(2):
                    nc.tensor.matmul(
                        out=psw[:], lhsT=awW_t[:, k : k + 1], rhs=h2T[k][:, sl],
                        start=(k == 0), stop=(k == 1),
                    )
                nc.scalar.activation(
                    out=wb_row[:, sl], in_=psw[:], func=Sigmoid,
                    bias=vecs_t[0:1, 16:17],
                )
            if debug:
                nc.sync.dma_start(out=dbg["wdbg"][:, :], in_=wb_row[:])
            for m in range(2):
                for n in range(NPAD // 512):
                    sl = slice(n * 512, (n + 1) * 512)
                    psb = p_w.tile([128, 512], F32, tag="wps")
                    nc.tensor.matmul(
                        out=psb[:], lhsT=ones1[:], rhs=wb_row[:, sl],
                        start=True, stop=True,
                    )
                    nc.vector.tensor_tensor(
                        out=hw_t[:, sl], in0=h2T[m][:, sl], in1=psb[:], op=AL.mult
                    )
                gfs = scr.tile([128, NSEG], F32, tag="gfs")
                nc.vector.reduce_sum(
                    out=gfs[:].unsqueeze(2),
                    in_=hw_t[:].rearrange("p (g s) -> p g s", s=Sg),
                    axis=AX,
                )
                nc.scalar.activation(out=z_t[m][:], in_=gfs[:], func=Copy)
                nc.vector.tensor_tensor(
                    out=hw_t[:], in0=h2T[m][:], in1=negm_t[:], op=AL.add
                )
                gfm = scr.tile([128, NSEG], F32, tag="gfm")
                nc.vector.reduce_max(
                    out=gfm[:].unsqueeze(2),
                    in_=hw_t[:].rearrange("p (g s) -> p g s", s=Sg),
                    axis=AX,
                )
                nc.scalar.activation(out=z_t[2 + m][:], in_=gfm[:], func=Copy)

            # ---------- phase 8: combined head ----------
            for m in range(8):
                ps = p_sm.tile([128, GPC], F32, tag="sm")
                for k in range(8):
                    wt = wp.tile([128, 128], BF16, tag="wt")
                    nc.sync.dma_start(out=wt[:], in_=cW1D[m * 8 + k, :, :])
                    nc.tensor.matmul(
                        out=ps[:], lhsT=wt[:], rhs=z_t[k][:],
                        start=(k == 0), stop=(k == 7),
                    )
                nc.scalar.activation(
                    out=c1_t[m][:], in_=ps[:], func=Relu,
                    bias=vecs_t[:, 30 + m : 31 + m],
                )
            for m in range(4):
                ps = p_sm.tile([128, GPC], F32, tag="sm")
                for k in range(8):
                    wt = wp.tile([128, 128], BF16, tag="wt")
                    nc.sync.dma_start(out=wt[:], in_=cW2D[m * 8 + k, :, :])
                    nc.tensor.matmul(
                        out=ps[:], lhsT=wt[:], rhs=c1_t[k][:],
                        start=(k == 0), stop=(k == 7),
                    )
                nc.scalar.activation(
                    out=c2_t[m][:], in_=ps[:], func=Relu,
                    bias=vecs_t[:, 38 + m : 39 + m],
                )
            ps0 = p_sm.tile([1, GPC], F32, tag="sm", name="ps0")
            for k in range(4):
                nc.tensor.matmul(
                    out=ps0[:], lhsT=cW3_t[:, k : k + 1], rhs=c2_t[k][:],
                    start=(k == 0), stop=(k == 3),
                )
            res_sb = scr.tile([1, GPC], F32, tag="res")
            nc.vector.tensor_scalar(
                out=res_sb[:], in0=ps0[:], scalar1=vecs_t[0:1, 17:18],
                scalar2=None, op0=AL.add,
            )
            nc.sync.dma_start(out=outD[:, :], in_=res_sb[:])

            if debug:
                nc.sync.dma_start(out=dbg["agg2dbg"][:, :], in_=aggH2[0][:])
                nc.sync.dma_start(out=dbg["h2dbg"][:, :], in_=h2T[0][:])
                zcat = scr.tile([128, 8 * GPC], BF16, tag="zcat")
                for i in range(8):
                    nc.vector.tensor_copy(
                        out=zcat[:, i * GPC : (i + 1) * GPC], in_=z_t[i][:]
                    )
                nc.sync.dma_start(out=dbg["zdbg"][:, :], in_=zcat[:])

    nc.compile()
    return nc


def kernel(**inputs):
    global LAST_EXEC_NS
    wkeys = [
        "W1", "b1", "rW1", "rb1", "g1", "be1",
        "W2", "b2", "rW2", "rb2", "g2", "be2",
        "aw_W", "aw_b", "rk_W1", "rk_b1", "rk_W2", "rk_b2",
        "c_W1", "c_b1", "c_W2", "c_b2", "c_W3", "c_b3",
    ]
    weights = {k: np.asarray(inputs[k]) for k in wkeys}
    cfg, per_core, shared = _host_prep(
        inputs["node_feats"], inputs["rdkit_feats"], inputs["src"],
        inputs["dst"], inputs["node_gid"], weights,
    )
    key = (cfg["Sg"], cfg["CH1"], cfg["CH2"], DEBUG)
    if key not in _prog_cache:
        _prog_cache[key] = _build(cfg, DEBUG)
    nc = _prog_cache[key]

    in_maps = []
    for c in range(NCORES):
        m = dict(shared)
        m.update(per_core[c])
        in_maps.append(m)

    res = run_bass_kernel_spmd(
        nc, in_maps, core_ids=list(range(NCORES)), trace=TRACE
    )
    LAST_EXEC_NS = res.exec_time_ns
    out = np.zeros((N_GRAPHS, 1), np.float32)
    for c in range(NCORES):
        out[c * GPC : (c + 1) * GPC, 0] = np.asarray(
            res.results[c]["out"], np.float32
        ).reshape(-1)
    if DEBUG:
        kernel.last_debug = [res.results[c] for c in range(NCORES)]
        kernel.last_cfg = cfg
        kernel.last_percore = per_core
    return out
